# revision 12
# baseline (speedup 1.0000x reference)
"""Distributed Trainium2 Bass kernel for GQA causal attention
(S=2048, DIM=4096, NH=32, NKV=8, HD=128), tensor-parallel over heads on 8
NeuronCores.

Per-core program (core c owns q-heads 4c..4c+3 and kv-head c):
  1. QKV projection, m-outer: each 128-row output block runs its full
     32-tile contraction before the next starts (2 PSUM banks, no
     bank-pressure stalls); host supplies contiguous [p, m, kb, col]
     weight layout and [p, si, kb, s] x layout for wide DMA lines.
  2. RoPE via signed pair-permutation matmul (f32r, full speed) + DVE
     combine, injected into the next m-block's matmul stream.
  3. v transposed via DMA-transpose XBAR (no PE/DVE cost).
  4. Causal attention, scores-transposed layout, depth-1 software
     pipeline (sT/exp of block i+1 emitted before pv of block i).
     Diagonal kv-blocks use a shrunken moving window [128d, 512) and a
     single shared [128,128] triangular mask applied in place.
     Denominator rides the 65th stationary column of v (ones).
  5. Per-(qt,head) normalize: reciprocal on [1,512], broadcast across
     partitions by a stride-0 DMA replication; per-head AllGather
     (16 x 1MB) issued eagerly.
  6. Output projection: wo input-dim permuted head-major on host; per
     (si, head-group) 32 matmuls interleaved into attention at head
     boundaries, partial sums accumulated on DVE into SBUF.
"""

import sys

sys.path.insert(0, "/opt/trn_rl_repo")

import numpy as np
import ml_dtypes

import concourse.bass as bass
import concourse.mybir as mybir
import concourse.tile as tile
from concourse import bacc
from concourse import bass_utils
from concourse.bass import broadcast_tensor_aps

S, DIM = 2048, 4096
NH, NKV, HD = 32, 8, 128
NCORES = 8
QH = NH // NCORES  # 4 q heads per core
KT = DIM // 128  # 32 contraction tiles
ST = S // 512  # 4 sequence tiles of 512
SCALE = 1.0 / float(np.sqrt(HD))

BF = mybir.dt.bfloat16
F32 = mybir.dt.float32
F32R = mybir.dt.float32r
ALU = mybir.AluOpType
ACTF = mybir.ActivationFunctionType

USE_DMA_TRANSPOSE = False


def r32(ap):
    return ap.bitcast(F32R)


def block_list(qt):
    """Block order for one (head, qt): diagonal d=0 first (full width,
    start flag), then off-diagonals, then shrinking diagonals d=1..3.
    Entries: (j, qoff, width, is_diag)."""
    bl = [(4 * qt, 0, 512, True)]
    bl += [(j, 0, 512, False) for j in range(4 * qt)]
    bl += [(4 * qt + d, 128 * d, 512 - 128 * d, True) for d in (1, 2, 3)]
    return bl


# outproj groups (si, h) emitted after attention head (qt, h') completes.
OP_SCHED = {
    (3, 3): [(3, 0)],
    (2, 0): [(3, 1)],
    (2, 1): [(3, 2)],
    (2, 2): [(3, 3)],
    (2, 3): [(2, 0)],
    (1, 0): [(2, 1)],
    (1, 1): [(2, 2)],
    (1, 2): [(2, 3)],
    (1, 3): [(1, 0)],
    (0, 0): [(1, 1)],
    (0, 1): [(1, 2)],
    (0, 2): [(1, 3)],
    (0, 3): [(0, 0)],
    "end": [(0, 1), (0, 2), (0, 3)],
}


def build_nc():
    nc = bacc.Bacc(
        "TRN2",
        target_bir_lowering=False,
        debug=False,
        enable_asserts=True,
        num_devices=NCORES,
    )

    xt = nc.dram_tensor("xt", [128, ST * KT * 512], BF, kind="ExternalInput").ap()
    wqkvt = nc.dram_tensor("wqkvt", [128, 6 * KT * 128], BF, kind="ExternalInput").ap()
    wot = nc.dram_tensor("wot", [128, KT * 512], BF, kind="ExternalInput").ap()
    cost = nc.dram_tensor("cost", [128, S], F32, kind="ExternalInput").ap()
    sint = nc.dram_tensor("sint", [128, S], F32, kind="ExternalInput").ap()
    trit = nc.dram_tensor("trit", [128, 128], BF, kind="ExternalInput").ap()
    rpermt = nc.dram_tensor("rpermt", [128, 128], F32R, kind="ExternalInput").ap()
    identt = nc.dram_tensor("identt", [128, 128], F32, kind="ExternalInput").ap()
    onescolt = nc.dram_tensor("onescolt", [1, 128], F32R, kind="ExternalInput").ap()
    outt = nc.dram_tensor("outt", [512, S], F32, kind="ExternalOutput").ap()

    xt_r = xt.rearrange("p (si kb s) -> p si kb s", si=ST, kb=KT)
    wqkvt_r = wqkvt.rearrange("p (m kb c) -> p m kb c", m=6, kb=KT)
    wot_r = wot.rearrange("p (kb c) -> p kb c", kb=KT)
    outt_r = outt.rearrange("(oc p) s -> p oc s", p=128)

    with tile.TileContext(nc) as tc:
        with (
            tc.tile_pool(name="const", bufs=1) as const,
            tc.tile_pool(name="qkvsb", bufs=1) as qkvsb,
            tc.tile_pool(name="ps", bufs=8, space="PSUM") as ps,
            tc.tile_pool(name="dram", bufs=1, space="DRAM") as dram,
        ):
            tri_sb = const.tile([128, 128], BF)
            rperm_sb = const.tile([128, 128], F32R)
            ident_sb = const.tile([128, 128], F32)
            onescol_sb = const.tile([1, 128], F32R)

            # persistent activations, attention operands in bf16
            q_sb = qkvsb.tile([128, QH, S], BF)  # rope'd qT, head-major
            k_sb = qkvsb.tile([128, S], BF)  # rope'd kT
            # v, block-transposed, 130 cols per kv-block: [v(128) | ones,pad]
            v_sb = qkvsb.tile([128, (S // 128) * 130], BF)
            wo_sb = qkvsb.tile([128, KT, 512], BF)

            def A(name):
                return ps.tile([128, 512], F32, tag="A", bufs=2, name=name)

            def B(name, shape=(128, 512)):
                return ps.tile(list(shape), F32, tag="B", bufs=2, name=name)

            def C(name, shape=(128, 512)):
                return ps.tile(list(shape), F32, tag="C", bufs=2, name=name)

            # ---------------- phase 1: QKV projections + RoPE ----------------
            with (
                tc.tile_pool(name="wqkv", bufs=1) as wqkv,
                tc.tile_pool(name="xs", bufs=1) as xs,
                tc.tile_pool(name="stg", bufs=1) as stg,
                tc.tile_pool(name="rope", bufs=1) as ropep,
            ):
                cos_sb = ropep.tile([128, S], F32)
                sin_sb = ropep.tile([128, S], F32)

                w_sb = wqkv.tile([128, 6, KT, 128], BF)
                for m in range(6):
                    nc.sync.dma_start(w_sb[:, m], wqkvt_r[:, m])

                x_tiles = {}

                def load_x(si):
                    t = xs.tile([128, KT, 512], BF, tag="x", bufs=2, name=f"x{si}")
                    for k2 in range(KT // 2):
                        nc.sync.dma_start(
                            t[:, 2 * k2 : 2 * k2 + 2, :],
                            xt_r[:, si, 2 * k2 : 2 * k2 + 2, :],
                        )
                    x_tiles[si] = t

                load_x(0)
                # consts + wo on the scalar queue (idle during phase 1)
                nc.scalar.dma_start(cos_sb, cost)
                nc.scalar.dma_start(sin_sb, sint)
                nc.scalar.dma_start(tri_sb, trit)
                nc.scalar.dma_start(rperm_sb, rpermt)
                nc.scalar.dma_start(ident_sb, identt)
                nc.scalar.dma_start(onescol_sb, onescolt)
                for k4 in range(KT // 4):
                    nc.scalar.dma_start(
                        wo_sb[:, 4 * k4 : 4 * k4 + 4, :],
                        wot_r[:, 4 * k4 : 4 * k4 + 4, :],
                    )
                # ones columns of v (memset once; strided AP)
                v_r = v_sb.rearrange("p (b c) -> p b c", c=130)
                nc.vector.memset(v_r[:, :, 128:130], 1.0)

                def make_rope(si, m, src_ps):
                    s0 = 512 * si

                    def fire():
                        stage = stg.tile([128, 512], F32, tag="stage", bufs=2)
                        nc.vector.tensor_copy(r32(stage), src_ps)
                        rot = C(f"rot_{si}_{m}")
                        nc.tensor.matmul(rot, rperm_sb, r32(stage))
                        dst = (
                            q_sb[:, m, s0 : s0 + 512]
                            if m < QH
                            else k_sb[:, s0 : s0 + 512]
                        )
                        t1 = stg.tile([128, 512], F32, tag="t1", bufs=2)
                        nc.vector.tensor_tensor(
                            t1, stage, cos_sb[:, s0 : s0 + 512], ALU.mult
                        )
                        t2 = stg.tile([128, 512], F32, tag="t2", bufs=2)
                        nc.vector.tensor_tensor(
                            t2, rot, sin_sb[:, s0 : s0 + 512], ALU.mult
                        )
                        nc.vector.tensor_tensor(dst, t1, t2, ALU.add)

                    return fire

                def make_v(si, src_ps):
                    def fire():
                        vdt = BF if USE_DMA_TRANSPOSE else F32
                        vstage = stg.tile([128, 512], vdt, tag="vstage", bufs=2)
                        nc.vector.tensor_copy(vstage, src_ps)
                        for jj in range(4):
                            j = 4 * si + jj
                            blk = vstage[:, 128 * jj : 128 * (jj + 1)]
                            dst = v_sb[:, 130 * j : 130 * j + 128]
                            if USE_DMA_TRANSPOSE:
                                nc.sync.dma_start_transpose(dst, blk)
                            else:
                                vt_ps = C(f"vt_{si}_{jj}", (128, 128))
                                nc.tensor.transpose(vt_ps, blk, ident_sb)
                                nc.vector.tensor_copy(dst, vt_ps)

                    return fire

                pending = None
                for si in range(ST):
                    for m in range(6):
                        acc = A(f"qkv_{si}_{m}")
                        for k in range(KT):
                            if k == 3 and pending is not None:
                                pending()
                                pending = None
                            if k == 8 and m == 1 and si < ST - 1:
                                load_x(si + 1)
                            nc.tensor.matmul(
                                acc,
                                w_sb[:, m, k, :],
                                x_tiles[si][:, k, :],
                                start=(k == 0),
                                stop=(k == KT - 1),
                            )
                        pending = make_rope(si, m, acc) if m < 5 else make_v(si, acc)
                    if si == ST - 1:
                        pending()  # v of last si, inline
                        pending = None

            # ---------------- phase 2: attention + normalize + outproj ------
            y_bounce = {}
            y_gather = {}
            for qt in range(ST):
                for h in range(QH):
                    y_bounce[(qt, h)] = dram.tile(
                        [128, 512], BF, tag=f"yb{qt}_{h}", name=f"ybounce{qt}_{h}"
                    )
                    y_gather[(qt, h)] = dram.tile(
                        [NCORES * 128, 512],
                        BF,
                        addr_space="Shared",
                        tag=f"yg{qt}_{h}",
                        name=f"ygather{qt}_{h}",
                    )

            with (
                tc.tile_pool(name="pp", bufs=1) as pp,
                tc.tile_pool(name="nrm", bufs=1) as nrm,
                tc.tile_pool(name="ys", bufs=1) as ys,
                tc.tile_pool(name="osb", bufs=1) as osb,
            ):
                osb_acc = {}

                def emit_outproj_group(si, h):
                    ystrip = ys.tile(
                        [128, NCORES, 512], BF, tag="ystrip", bufs=2,
                        name=f"ystrip_{si}_{h}",
                    )
                    ysrc = y_gather[(si, h)].opt().rearrange(
                        "(c p) q -> p c q", p=128
                    )
                    nc.sync.dma_start(ystrip, ysrc)
                    if h == 0:
                        osb_acc[si] = osb.tile(
                            [128, 4, 512], F32, tag="osb", bufs=2, name=f"osb{si}"
                        )
                    for pair in ((0, 1), (2, 3)):
                        pps = {
                            oc: B(f"op_{si}_{h}_{oc}") for oc in pair
                        }
                        for ci in range(NCORES):
                            for oc in pair:
                                nc.tensor.matmul(
                                    pps[oc],
                                    wo_sb[:, 8 * h + ci, 128 * oc : 128 * (oc + 1)],
                                    ystrip[:, ci, :],
                                    start=(ci == 0),
                                    stop=(ci == NCORES - 1),
                                )
                        for oc in pair:
                            if h == 0:
                                nc.vector.tensor_copy(
                                    osb_acc[si][:, oc, :], pps[oc]
                                )
                            else:
                                nc.vector.tensor_tensor(
                                    osb_acc[si][:, oc, :],
                                    osb_acc[si][:, oc, :],
                                    pps[oc],
                                    ALU.add,
                                )
                    if h == QH - 1:
                        s0 = 512 * si
                        nc.sync.dma_start(
                            outt_r[:, :, s0 : s0 + 512], osb_acc[si]
                        )

                def normalize_tail(st):
                    yraw, den, h, qt = st
                    rec = nrm.tile([1, 512], F32R, tag="rec", bufs=2)
                    with nc.allow_low_precision(reason="f32r reciprocal"):
                        nc.vector.reciprocal(rec, den)
                    bc_ps = ps.tile([128, 512], F32, tag="bc", bufs=1,
                                    name=f"bc_{qt}_{h}")
                    nc.tensor.matmul(bc_ps, onescol_sb, rec)
                    yn = nrm.tile([128, 512], BF, tag="yn", bufs=2)
                    nc.vector.tensor_tensor(yn, yraw, bc_ps, ALU.mult)
                    nc.sync.dma_start(y_bounce[(qt, h)], yn)
                    nc.gpsimd.collective_compute(
                        "AllGather",
                        ALU.bypass,
                        ins=[y_bounce[(qt, h)].opt()],
                        outs=[y_gather[(qt, h)].opt()],
                        replica_groups=[list(range(NCORES))],
                    )

                # flat task list over (qt desc, h, blocks) with depth-1
                # lookahead: sT/exp of task i+1 emitted before pv of task i.
                tasks = []
                for qt in reversed(range(ST)):
                    for h in range(QH):
                        bl = block_list(qt)
                        for bi, blk in enumerate(bl):
                            tasks.append((qt, h, blk, bi == len(bl) - 1))

                state = {}  # (qt,h) -> (ya_ps, yb_ps)
                pend_norm = None

                def emit_spe(i):
                    qt, h, (j, qoff, w, diag), _ = tasks[i]
                    s0 = 512 * qt
                    sT = C(f"sT_{i}")
                    nc.tensor.matmul(
                        sT[:, 0:w],
                        k_sb[:, 128 * j : 128 * (j + 1)],
                        q_sb[:, h, s0 + qoff : s0 + qoff + w],
                    )
                    p = pp.tile([128, 512], BF, tag="p", bufs=4, name=f"p_{i}")
                    nc.scalar.activation(p[:, 0:w], sT[:, 0:w], ACTF.Exp, scale=SCALE)
                    if diag:
                        nc.vector.tensor_tensor(
                            p[:, 0:128], p[:, 0:128], tri_sb, ALU.mult
                        )
                    return p

                def emit_pv(i, p):
                    nonlocal pend_norm
                    qt, h, (j, qoff, w, diag), last = tasks[i]
                    if (qt, h) not in state:
                        state[(qt, h)] = (
                            ps.tile([64, 512], F32, tag="A", bufs=2,
                                    name=f"ya_{qt}_{h}"),
                            ps.tile([65, 512], F32, tag="A", bufs=2,
                                    name=f"yb_{qt}_{h}"),
                        )
                    ya_ps, yb_ps = state[(qt, h)]
                    first = (qoff == 0 and diag)  # d=0 block is emitted first
                    nc.tensor.matmul(
                        ya_ps[:, qoff : qoff + w],
                        v_sb[:, 130 * j : 130 * j + 64],
                        p[:, 0:w],
                        start=first,
                        stop=last,
                        skip_group_check=True,
                    )
                    nc.tensor.matmul(
                        yb_ps[:, qoff : qoff + w],
                        v_sb[:, 130 * j + 64 : 130 * j + 129],
                        p[:, 0:w],
                        start=first,
                        stop=last,
                        skip_group_check=True,
                    )
                    if last:
                        # immediate psum drains (free the banks)
                        yraw = nrm.tile([128, 512], F32, tag="yraw", bufs=2,
                                        name=f"yraw_{qt}_{h}")
                        nc.vector.tensor_copy(yraw[0:64, :], ya_ps)
                        nc.vector.tensor_copy(yraw[64:128, :], yb_ps[0:64, :])
                        den = nrm.tile([1, 512], F32, tag="den", bufs=2,
                                       name=f"den_{qt}_{h}")
                        nc.vector.tensor_copy(den, yb_ps[64:65, :])
                        if pend_norm is not None:
                            normalize_tail(pend_norm)
                        pend_norm = (yraw, den, h, qt)
                        if qt == 0 and h == QH - 1:
                            normalize_tail(pend_norm)  # eager final
                            pend_norm = None
                        for g in OP_SCHED.get((qt, h), []):
                            emit_outproj_group(*g)

                prev_p = None
                for i in range(len(tasks)):
                    p = emit_spe(i)
                    if prev_p is not None:
                        emit_pv(i - 1, prev_p)
                    prev_p = p
                emit_pv(len(tasks) - 1, prev_p)
                for g in OP_SCHED["end"]:
                    emit_outproj_group(*g)

    nc.compile()
    return nc


def make_in_maps(x, freqs_cis, wq, wk, wv, wo):
    f32 = np.float32
    bf = ml_dtypes.bfloat16
    xT = np.ascontiguousarray(x.T)  # [DIM, S]
    xt2 = (
        xT.reshape(KT, 128, ST, 512).transpose(1, 2, 0, 3).reshape(128, -1)
    ).astype(bf)
    cos = np.ascontiguousarray(np.repeat(freqs_cis[:, :, 0].T, 2, axis=0)).astype(f32)
    sin = np.ascontiguousarray(np.repeat(freqs_cis[:, :, 1].T, 2, axis=0)).astype(f32)
    kvi = np.arange(128)[:, None]
    qi = np.arange(128)[None, :]
    tri = (kvi <= qi).astype(f32).astype(bf)  # [128,128]
    rperm = np.zeros((128, 128), f32)
    for r in range(64):
        rperm[2 * r, 2 * r + 1] = -1.0
        rperm[2 * r + 1, 2 * r] = 1.0
    rpermT = np.ascontiguousarray(rperm.T)
    ident = np.eye(128, dtype=f32)
    onescol = np.ones((1, 128), f32)

    # head-major input-dim permutation for wo
    idx = np.arange(DIM)
    hh, rem = idx // 1024, idx % 1024
    cc, hd = rem // 128, rem % 128
    perm = (4 * cc + hh) * 128 + hd

    in_maps = []
    for c in range(NCORES):
        W = np.concatenate(
            [
                wq[512 * c : 512 * (c + 1), :],
                wk[128 * c : 128 * (c + 1), :],
                wv[128 * c : 128 * (c + 1), :],
            ],
            axis=0,
        )  # [768, DIM]
        WT = np.ascontiguousarray(W.T)  # [DIM, 768]
        wqkv2 = (
            WT.reshape(KT, 128, 6, 128).transpose(1, 2, 0, 3).reshape(128, -1)
        ).astype(bf)
        wo_c = wo[512 * c : 512 * (c + 1), :][:, perm]  # [512, DIM] permuted
        WoT = np.ascontiguousarray(wo_c.T)  # [DIM, 512]
        wo2 = (
            WoT.reshape(KT, 128, 4, 128).transpose(1, 0, 2, 3).reshape(128, -1)
        ).astype(bf)
        in_maps.append(
            {
                "xt": np.ascontiguousarray(xt2),
                "wqkvt": np.ascontiguousarray(wqkv2),
                "wot": np.ascontiguousarray(wo2),
                "cost": cos,
                "sint": sin,
                "trit": np.ascontiguousarray(tri),
                "rpermt": rpermT,
                "identt": ident,
                "onescolt": onescol,
            }
        )
    return in_maps


def install_ntff_hook():
    """Inject the missing ``antenv.axon_hooks`` module backed by ctypes calls
    into libaxon_pjrt.so, enabling run_bass_kernel_spmd(trace=True) under
    axon. Also neuter upload_artifacts (no artifact bucket here)."""
    import sys as _sys
    import types
    import ctypes
    import contextlib

    if "antenv.axon_hooks" in _sys.modules:
        return
    so_path = "/opt/axon/libaxon_pjrt.so"
    lib = ctypes.CDLL(so_path)
    lib.axon_start_nrt_profile.argtypes = [
        ctypes.POINTER(ctypes.c_int64),
        ctypes.c_size_t,
    ]
    lib.axon_start_nrt_profile.restype = ctypes.c_int64
    lib.axon_stop_nrt_profile.argtypes = [ctypes.c_char_p]
    lib.axon_stop_nrt_profile.restype = ctypes.c_int64

    @contextlib.contextmanager
    def _hook(output_dir, device_ids):
        import jax

        jax.devices()
        if device_ids:
            ids = (ctypes.c_int64 * len(device_ids))(*device_ids)
            rc = lib.axon_start_nrt_profile(ids, len(device_ids))
        else:
            rc = lib.axon_start_nrt_profile(None, 0)
        if rc != 0:
            raise RuntimeError(f"axon_start_nrt_profile rc={rc}")
        try:
            yield
        finally:
            n = lib.axon_stop_nrt_profile(str(output_dir).encode())
            print(f"ntff profile: {n} file(s) written to {output_dir}")

    mod = types.ModuleType("antenv.axon_hooks")
    mod.get_axon_ntff_profile_hook = lambda: _hook
    mod.set_axon_ntff_profile_hook = lambda h: None
    _sys.modules["antenv.axon_hooks"] = mod
    import antenv

    antenv.axon_hooks = mod
    bass_utils.upload_artifacts = lambda tmpdir: tmpdir


def run(x, freqs_cis, wq, wk, wv, wo, trace=False, trace_kwargs=None):
    if trace:
        install_ntff_hook()
    nc = build_nc()
    in_maps = make_in_maps(x, freqs_cis, wq, wk, wv, wo)
    res = bass_utils.run_bass_kernel_spmd(
        nc,
        in_maps,
        core_ids=list(range(NCORES)),
        trace=trace,
        **(trace_kwargs or {}),
    )
    outs = [r["outt"] for r in res.results]  # each [512, S] = outT slice
    full = np.concatenate([np.asarray(o).T for o in outs], axis=1).astype(np.float32)
    return full, res


def kernel(x, freqs_cis, wq, wk, wv, wo):
    full, _ = run(
        np.asarray(x, np.float32),
        np.asarray(freqs_cis, np.float32),
        np.asarray(wq, np.float32),
        np.asarray(wk, np.float32),
        np.asarray(wv, np.float32),
        np.asarray(wo, np.float32),
    )
    return full


# revision 19
# speedup vs baseline: 1.0682x; 1.0682x over previous
"""Distributed Trainium2 Bass kernel for GQA causal attention
(S=2048, DIM=4096, NH=32, NKV=8, HD=128), tensor-parallel over heads on 8
NeuronCores.

Per-core program (core c owns q-heads 4c..4c+3 and kv-head c):
  1. QKV projection, m-outer: each 128-row output block runs its full
     32-tile contraction before the next starts (2 PSUM banks, no
     bank-pressure stalls); host supplies contiguous [p, m, kb, col]
     weight layout and [p, si, kb, s] x layout for wide DMA lines.
  2. RoPE via signed pair-permutation matmul (f32r, full speed) + DVE
     combine, injected into the next m-block's matmul stream.
  3. v transposed via DMA-transpose XBAR (no PE/DVE cost).
  4. Causal attention, scores-transposed layout, depth-1 software
     pipeline (sT/exp of block i+1 emitted before pv of block i).
     Diagonal kv-blocks use a shrunken moving window [128d, 512) and a
     single shared [128,128] triangular mask applied in place.
     Denominator rides the 65th stationary column of v (ones).
  5. Per-(qt,head) normalize: reciprocal on [1,512], broadcast across
     partitions by a stride-0 DMA replication; per-head AllGather
     (16 x 1MB) issued eagerly.
  6. Output projection: wo input-dim permuted head-major on host; per
     (si, head-group) 32 matmuls interleaved into attention at head
     boundaries, partial sums accumulated on DVE into SBUF.
"""

import sys

sys.path.insert(0, "/opt/trn_rl_repo")

import numpy as np
import ml_dtypes

import concourse.bass as bass
import concourse.mybir as mybir
import concourse.tile as tile
from concourse import bacc
from concourse import bass_utils
from concourse.bass import broadcast_tensor_aps

S, DIM = 2048, 4096
NH, NKV, HD = 32, 8, 128
NCORES = 8
QH = NH // NCORES  # 4 q heads per core
KT = DIM // 128  # 32 contraction tiles
ST = S // 512  # 4 sequence tiles of 512
SCALE = 1.0 / float(np.sqrt(HD))

BF = mybir.dt.bfloat16
F32 = mybir.dt.float32
F32R = mybir.dt.float32r
ALU = mybir.AluOpType
ACTF = mybir.ActivationFunctionType

USE_DMA_TRANSPOSE = False


def r32(ap):
    return ap.bitcast(F32R)


def block_list(qt):
    """Block order for one (head, qt): diagonal d=0 first (full width,
    start flag), then off-diagonals, then shrinking diagonals d=1..3.
    Entries: (j, qoff, width, is_diag)."""
    bl = [(4 * qt, 0, 512, True)]
    bl += [(j, 0, 512, False) for j in range(4 * qt)]
    bl += [(4 * qt + d, 128 * d, 512 - 128 * d, True) for d in (1, 2, 3)]
    return bl


# outproj groups (si, half, core-group) emitted after attention head
# (qt, h) completes; group needs AllGather(si, half) done.
OP_SCHED = {
    (2, 0): [(3, 0, 0)],
    (2, 1): [(3, 0, 1)],
    (2, 2): [(3, 1, 0)],
    (2, 3): [(3, 1, 1)],
    (1, 0): [(2, 0, 0)],
    (1, 1): [(2, 0, 1)],
    (1, 2): [(2, 1, 0)],
    (1, 3): [(2, 1, 1)],
    (0, 0): [(1, 0, 0)],
    (0, 1): [(1, 0, 1)],
    (0, 2): [(1, 1, 0)],
    (0, 3): [(1, 1, 1)],
    "end": [(0, 0, 0), (0, 0, 1), (0, 1, 0), (0, 1, 1)],
}


def build_nc():
    nc = bacc.Bacc(
        "TRN2",
        target_bir_lowering=False,
        debug=False,
        enable_asserts=True,
        num_devices=NCORES,
    )

    xt = nc.dram_tensor("xt", [128, ST * KT * 512], BF, kind="ExternalInput").ap()
    wqkvt = nc.dram_tensor("wqkvt", [128, 6 * KT * 128], BF, kind="ExternalInput").ap()
    wot = nc.dram_tensor("wot", [128, KT * 512], BF, kind="ExternalInput").ap()
    cost = nc.dram_tensor("cost", [128, S], F32, kind="ExternalInput").ap()
    sint = nc.dram_tensor("sint", [128, S], F32, kind="ExternalInput").ap()
    trit = nc.dram_tensor("trit", [128, 128], BF, kind="ExternalInput").ap()
    rpermt = nc.dram_tensor("rpermt", [128, 128], F32R, kind="ExternalInput").ap()
    identt = nc.dram_tensor("identt", [128, 128], F32, kind="ExternalInput").ap()
    onescolt = nc.dram_tensor("onescolt", [1, 128], F32R, kind="ExternalInput").ap()
    outt = nc.dram_tensor("outt", [512, S], F32, kind="ExternalOutput").ap()

    xt_r = xt.rearrange("p (si kb s) -> p si kb s", si=ST, kb=KT)
    wqkvt_r = wqkvt.rearrange("p (m kb c) -> p m kb c", m=6, kb=KT)
    wot_r = wot.rearrange("p (kb c) -> p kb c", kb=KT)
    outt_r = outt.rearrange("(oc p) s -> p oc s", p=128)

    with tile.TileContext(nc) as tc:
        with (
            tc.tile_pool(name="const", bufs=1) as const,
            tc.tile_pool(name="qkvsb", bufs=1) as qkvsb,
            tc.tile_pool(name="ps", bufs=8, space="PSUM") as ps,
            tc.tile_pool(name="dram", bufs=1, space="DRAM") as dram,
        ):
            tri_sb = const.tile([128, 128], BF)
            rperm_sb = const.tile([128, 128], F32R)
            ident_sb = const.tile([128, 128], F32)
            onescol_sb = const.tile([1, 128], F32R)

            # persistent activations, attention operands in bf16
            q_sb = qkvsb.tile([128, QH, S], BF)  # rope'd qT, head-major
            k_sb = qkvsb.tile([128, S], BF)  # rope'd kT
            # v, block-transposed, 130 cols per kv-block: [v(128) | ones,pad]
            v_sb = qkvsb.tile([128, (S // 128) * 130], BF)
            wo_sb = qkvsb.tile([128, KT, 512], BF)

            def A(name):
                return ps.tile([128, 512], F32, tag="A", bufs=2, name=name)

            def B(name, shape=(128, 512)):
                return ps.tile(list(shape), F32, tag="B", bufs=2, name=name)

            def C(name, shape=(128, 512)):
                return ps.tile(list(shape), F32, tag="C", bufs=2, name=name)

            # ---------------- phase 1: QKV projections + RoPE ----------------
            with (
                tc.tile_pool(name="wqkv", bufs=1) as wqkv,
                tc.tile_pool(name="xs", bufs=1) as xs,
                tc.tile_pool(name="stg", bufs=1) as stg,
                tc.tile_pool(name="rope", bufs=1) as ropep,
            ):
                cos_sb = ropep.tile([128, S], F32)
                sin_sb = ropep.tile([128, S], F32)

                w_sb = wqkv.tile([128, 6, KT, 128], BF)

                x_tiles = {}

                def load_x(si):
                    t = xs.tile([128, KT, 512], BF, tag="x", bufs=2, name=f"x{si}")
                    for k2 in range(KT // 2):
                        nc.sync.dma_start(
                            t[:, 2 * k2 : 2 * k2 + 2, :],
                            xt_r[:, si, 2 * k2 : 2 * k2 + 2, :],
                        )
                    x_tiles[si] = t

                # startup: w(m=0,1) + x(0) on sync; everything else on the
                # scalar queue so the first matmuls aren't blocked.
                nc.sync.dma_start(w_sb[:, 0], wqkvt_r[:, 0])
                nc.sync.dma_start(w_sb[:, 1], wqkvt_r[:, 1])
                load_x(0)
                nc.scalar.dma_start(cos_sb, cost)
                nc.scalar.dma_start(sin_sb, sint)
                nc.scalar.dma_start(tri_sb, trit)
                nc.scalar.dma_start(rperm_sb, rpermt)
                nc.scalar.dma_start(ident_sb, identt)
                nc.scalar.dma_start(onescol_sb, onescolt)
                for m in range(2, 6):
                    nc.scalar.dma_start(w_sb[:, m], wqkvt_r[:, m])
                for k4 in range(KT // 4):
                    nc.scalar.dma_start(
                        wo_sb[:, 4 * k4 : 4 * k4 + 4, :],
                        wot_r[:, 4 * k4 : 4 * k4 + 4, :],
                    )
                # ones columns of v (memset once; strided AP)
                v_r = v_sb.rearrange("p (b c) -> p b c", c=130)
                nc.vector.memset(v_r[:, :, 128:130], 1.0)

                def make_rope(si, m, src_ps):
                    s0 = 512 * si

                    def fire():
                        stage = stg.tile([128, 512], F32, tag="stage", bufs=2)
                        nc.vector.tensor_copy(r32(stage), src_ps)
                        rot = C(f"rot_{si}_{m}")
                        nc.tensor.matmul(rot, rperm_sb, r32(stage))
                        dst = (
                            q_sb[:, m, s0 : s0 + 512]
                            if m < QH
                            else k_sb[:, s0 : s0 + 512]
                        )
                        t1 = stg.tile([128, 512], F32, tag="t1", bufs=2)
                        nc.vector.tensor_tensor(
                            t1, stage, cos_sb[:, s0 : s0 + 512], ALU.mult
                        )
                        t2 = stg.tile([128, 512], F32, tag="t2", bufs=2)
                        nc.vector.tensor_tensor(
                            t2, rot, sin_sb[:, s0 : s0 + 512], ALU.mult
                        )
                        nc.vector.tensor_tensor(dst, t1, t2, ALU.add)

                    return fire

                def make_v(si, src_ps):
                    def fire():
                        vdt = BF if USE_DMA_TRANSPOSE else F32
                        vstage = stg.tile([128, 512], vdt, tag="vstage", bufs=2)
                        nc.vector.tensor_copy(vstage, src_ps)
                        for jj in range(4):
                            j = 4 * si + jj
                            blk = vstage[:, 128 * jj : 128 * (jj + 1)]
                            dst = v_sb[:, 130 * j : 130 * j + 128]
                            if USE_DMA_TRANSPOSE:
                                nc.sync.dma_start_transpose(dst, blk)
                            else:
                                vt_ps = C(f"vt_{si}_{jj}", (128, 128))
                                nc.tensor.transpose(vt_ps, blk, ident_sb)
                                nc.vector.tensor_copy(dst, vt_ps)

                    return fire

                pending = None
                for si in range(ST):
                    for m in range(6):
                        acc = A(f"qkv_{si}_{m}")
                        for k in range(KT):
                            if k == 3 and pending is not None:
                                pending()
                                pending = None
                            if k == 8 and m == 1 and si < ST - 1:
                                load_x(si + 1)
                            nc.tensor.matmul(
                                acc,
                                w_sb[:, m, k, :],
                                x_tiles[si][:, k, :],
                                start=(k == 0),
                                stop=(k == KT - 1),
                            )
                        pending = make_rope(si, m, acc) if m < 5 else make_v(si, acc)
                    if si == ST - 1:
                        pending()  # v of last si, inline
                        pending = None

            # ---------------- phase 2: attention + normalize + outproj ------
            # y gathered per (qt, head-half): each AllGather moves 2 heads.
            y_bounce = {}
            y_gather = {}
            for qt in range(ST):
                for hf in range(2):
                    y_bounce[(qt, hf)] = dram.tile(
                        [256, 512], BF, tag=f"yb{qt}_{hf}", name=f"ybounce{qt}_{hf}"
                    )
                    y_gather[(qt, hf)] = dram.tile(
                        [NCORES * 256, 512],
                        BF,
                        addr_space="Shared",
                        tag=f"yg{qt}_{hf}",
                        name=f"ygather{qt}_{hf}",
                    )

            with (
                tc.tile_pool(name="pp", bufs=1) as pp,
                tc.tile_pool(name="nrm", bufs=1) as nrm,
                tc.tile_pool(name="ys", bufs=1) as ys,
                tc.tile_pool(name="osb", bufs=1) as osb,
            ):
                osb_acc = {}
                group_cnt = {}

                def emit_outproj_group(si, hf, cg):
                    ystrip = ys.tile(
                        [128, 8, 512], BF, tag="ystrip", bufs=2,
                        name=f"ystrip_{si}_{hf}_{cg}",
                    )
                    ysrc = y_gather[(si, hf)].opt().rearrange(
                        "(ch p) q -> p ch q", p=128
                    )[:, 8 * cg : 8 * cg + 8, :]
                    nc.sync.dma_start(ystrip, ysrc)
                    n = group_cnt.get(si, 0)
                    group_cnt[si] = n + 1
                    if n == 0:
                        osb_acc[si] = osb.tile(
                            [128, 4, 512], F32, tag="osb", bufs=2, name=f"osb{si}"
                        )
                    for pair in ((0, 1), (2, 3)):
                        pps = {
                            oc: B(f"op_{si}_{hf}_{cg}_{oc}") for oc in pair
                        }
                        for ci in range(8):
                            # strip col ci = (core 4*cg + ci//2, head 2*hf + ci%2)
                            k = 4 * (4 * cg + ci // 2) + 2 * hf + ci % 2
                            for oc in pair:
                                nc.tensor.matmul(
                                    pps[oc],
                                    wo_sb[:, k, 128 * oc : 128 * (oc + 1)],
                                    ystrip[:, ci, :],
                                    start=(ci == 0),
                                    stop=(ci == 7),
                                )
                        for oc in pair:
                            if n == 0:
                                nc.vector.tensor_copy(
                                    osb_acc[si][:, oc, :], pps[oc]
                                )
                            else:
                                nc.vector.tensor_tensor(
                                    osb_acc[si][:, oc, :],
                                    osb_acc[si][:, oc, :],
                                    pps[oc],
                                    ALU.add,
                                )
                    if n == 3:
                        s0 = 512 * si
                        nc.sync.dma_start(
                            outt_r[:, :, s0 : s0 + 512], osb_acc[si]
                        )

                def normalize_tail(st):
                    yraw, den, h, qt = st
                    den_t = nrm.tile([128, 4], F32, tag="dent", bufs=2)
                    nc.sync.dma_start(den_t, den)
                    rec_t = nrm.tile([128, 4], F32R, tag="rect", bufs=2)
                    with nc.allow_low_precision(reason="f32r reciprocal"):
                        nc.vector.reciprocal(rec_t, den_t)
                    rec = nrm.tile([1, 512], F32R, tag="rec", bufs=2)
                    nc.sync.dma_start(rec, rec_t)
                    bc_ps = ps.tile([128, 512], F32, tag="bc", bufs=1,
                                    name=f"bc_{qt}_{h}")
                    nc.tensor.matmul(bc_ps, onescol_sb, rec)
                    yn = nrm.tile([128, 512], BF, tag="yn", bufs=2)
                    nc.vector.tensor_tensor(yn, yraw, bc_ps, ALU.mult)
                    hf, hh = h // 2, h % 2
                    nc.sync.dma_start(
                        y_bounce[(qt, hf)][128 * hh : 128 * (hh + 1), :], yn
                    )
                    if hh == 1:
                        nc.gpsimd.collective_compute(
                            "AllGather",
                            ALU.bypass,
                            ins=[y_bounce[(qt, hf)].opt()],
                            outs=[y_gather[(qt, hf)].opt()],
                            replica_groups=[list(range(NCORES))],
                        )

                # flat task list over (qt desc, h, blocks) with depth-1
                # lookahead: sT/exp of task i+1 emitted before pv of task i.
                tasks = []
                for qt in reversed(range(ST)):
                    for h in range(QH):
                        bl = block_list(qt)
                        for bi, blk in enumerate(bl):
                            tasks.append((qt, h, blk, bi == len(bl) - 1))

                state = {}  # (qt,h) -> (ya_ps, yb_ps)
                pend_norm = None

                def emit_spe(i):
                    qt, h, (j, qoff, w, diag), _ = tasks[i]
                    s0 = 512 * qt
                    sT = C(f"sT_{i}")
                    nc.tensor.matmul(
                        sT[:, 0:w],
                        k_sb[:, 128 * j : 128 * (j + 1)],
                        q_sb[:, h, s0 + qoff : s0 + qoff + w],
                    )
                    p = pp.tile([128, 512], BF, tag="p", bufs=4, name=f"p_{i}")
                    nc.scalar.activation(p[:, 0:w], sT[:, 0:w], ACTF.Exp, scale=SCALE)
                    if diag:
                        nc.vector.tensor_tensor(
                            p[:, 0:128], p[:, 0:128], tri_sb, ALU.mult
                        )
                    return p

                def emit_pv(i, p):
                    nonlocal pend_norm
                    qt, h, (j, qoff, w, diag), last = tasks[i]
                    if (qt, h) not in state:
                        state[(qt, h)] = (
                            ps.tile([64, 512], F32, tag="A", bufs=2,
                                    name=f"ya_{qt}_{h}"),
                            ps.tile([65, 512], F32, tag="A", bufs=2,
                                    name=f"yb_{qt}_{h}"),
                        )
                    ya_ps, yb_ps = state[(qt, h)]
                    first = (qoff == 0 and diag)  # d=0 block is emitted first
                    nc.tensor.matmul(
                        ya_ps[:, qoff : qoff + w],
                        v_sb[:, 130 * j : 130 * j + 64],
                        p[:, 0:w],
                        start=first,
                        stop=last,
                        skip_group_check=True,
                    )
                    nc.tensor.matmul(
                        yb_ps[:, qoff : qoff + w],
                        v_sb[:, 130 * j + 64 : 130 * j + 129],
                        p[:, 0:w],
                        start=first,
                        stop=last,
                        skip_group_check=True,
                    )
                    if last:
                        # immediate psum drains (free the banks)
                        yraw = nrm.tile([128, 512], F32, tag="yraw", bufs=2,
                                        name=f"yraw_{qt}_{h}")
                        nc.vector.tensor_copy(yraw[0:64, :], ya_ps)
                        nc.vector.tensor_copy(yraw[64:128, :], yb_ps[0:64, :])
                        den = nrm.tile([1, 512], F32, tag="den", bufs=2,
                                       name=f"den_{qt}_{h}")
                        nc.vector.tensor_copy(den, yb_ps[64:65, :])
                        if pend_norm is not None:
                            normalize_tail(pend_norm)
                        pend_norm = (yraw, den, h, qt)
                        if qt == 0 and h == QH - 1:
                            normalize_tail(pend_norm)  # eager final
                            pend_norm = None
                        for g in OP_SCHED.get((qt, h), []):
                            emit_outproj_group(*g)

                prev_p = None
                for i in range(len(tasks)):
                    p = emit_spe(i)
                    if prev_p is not None:
                        emit_pv(i - 1, prev_p)
                    prev_p = p
                emit_pv(len(tasks) - 1, prev_p)
                for g in OP_SCHED["end"]:
                    emit_outproj_group(*g)

    nc.compile()
    return nc


def make_in_maps(x, freqs_cis, wq, wk, wv, wo):
    f32 = np.float32
    bf = ml_dtypes.bfloat16
    xT = np.ascontiguousarray(x.T)  # [DIM, S]
    xt2 = (
        xT.reshape(KT, 128, ST, 512).transpose(1, 2, 0, 3).reshape(128, -1)
    ).astype(bf)
    cos = np.ascontiguousarray(np.repeat(freqs_cis[:, :, 0].T, 2, axis=0)).astype(f32)
    sin = np.ascontiguousarray(np.repeat(freqs_cis[:, :, 1].T, 2, axis=0)).astype(f32)
    kvi = np.arange(128)[:, None]
    qi = np.arange(128)[None, :]
    tri = (kvi <= qi).astype(f32).astype(bf)  # [128,128]
    rperm = np.zeros((128, 128), f32)
    for r in range(64):
        rperm[2 * r, 2 * r + 1] = -1.0
        rperm[2 * r + 1, 2 * r] = 1.0
    rpermT = np.ascontiguousarray(rperm.T)
    ident = np.eye(128, dtype=f32)
    onescol = np.ones((1, 128), f32)

    in_maps = []
    for c in range(NCORES):
        W = np.concatenate(
            [
                wq[512 * c : 512 * (c + 1), :],
                wk[128 * c : 128 * (c + 1), :],
                wv[128 * c : 128 * (c + 1), :],
            ],
            axis=0,
        )  # [768, DIM]
        WT = np.ascontiguousarray(W.T)  # [DIM, 768]
        wqkv2 = (
            WT.reshape(KT, 128, 6, 128).transpose(1, 2, 0, 3).reshape(128, -1)
        ).astype(bf)
        wo_c = wo[512 * c : 512 * (c + 1), :]  # [512, DIM]
        WoT = np.ascontiguousarray(wo_c.T)  # [DIM, 512]
        wo2 = (
            WoT.reshape(KT, 128, 4, 128).transpose(1, 0, 2, 3).reshape(128, -1)
        ).astype(bf)
        in_maps.append(
            {
                "xt": np.ascontiguousarray(xt2),
                "wqkvt": np.ascontiguousarray(wqkv2),
                "wot": np.ascontiguousarray(wo2),
                "cost": cos,
                "sint": sin,
                "trit": np.ascontiguousarray(tri),
                "rpermt": rpermT,
                "identt": ident,
                "onescolt": onescol,
            }
        )
    return in_maps


def install_ntff_hook():
    """Inject the missing ``antenv.axon_hooks`` module backed by ctypes calls
    into libaxon_pjrt.so, enabling run_bass_kernel_spmd(trace=True) under
    axon. Also neuter upload_artifacts (no artifact bucket here)."""
    import sys as _sys
    import types
    import ctypes
    import contextlib

    if "antenv.axon_hooks" in _sys.modules:
        return
    so_path = "/opt/axon/libaxon_pjrt.so"
    lib = ctypes.CDLL(so_path)
    lib.axon_start_nrt_profile.argtypes = [
        ctypes.POINTER(ctypes.c_int64),
        ctypes.c_size_t,
    ]
    lib.axon_start_nrt_profile.restype = ctypes.c_int64
    lib.axon_stop_nrt_profile.argtypes = [ctypes.c_char_p]
    lib.axon_stop_nrt_profile.restype = ctypes.c_int64

    @contextlib.contextmanager
    def _hook(output_dir, device_ids):
        import jax

        jax.devices()
        if device_ids:
            ids = (ctypes.c_int64 * len(device_ids))(*device_ids)
            rc = lib.axon_start_nrt_profile(ids, len(device_ids))
        else:
            rc = lib.axon_start_nrt_profile(None, 0)
        if rc != 0:
            raise RuntimeError(f"axon_start_nrt_profile rc={rc}")
        try:
            yield
        finally:
            n = lib.axon_stop_nrt_profile(str(output_dir).encode())
            print(f"ntff profile: {n} file(s) written to {output_dir}")

    mod = types.ModuleType("antenv.axon_hooks")
    mod.get_axon_ntff_profile_hook = lambda: _hook
    mod.set_axon_ntff_profile_hook = lambda h: None
    _sys.modules["antenv.axon_hooks"] = mod
    import antenv

    antenv.axon_hooks = mod
    bass_utils.upload_artifacts = lambda tmpdir: tmpdir


def run(x, freqs_cis, wq, wk, wv, wo, trace=False, trace_kwargs=None):
    if trace:
        install_ntff_hook()
    nc = build_nc()
    in_maps = make_in_maps(x, freqs_cis, wq, wk, wv, wo)
    res = bass_utils.run_bass_kernel_spmd(
        nc,
        in_maps,
        core_ids=list(range(NCORES)),
        trace=trace,
        **(trace_kwargs or {}),
    )
    outs = [r["outt"] for r in res.results]  # each [512, S] = outT slice
    full = np.concatenate([np.asarray(o).T for o in outs], axis=1).astype(np.float32)
    return full, res


def kernel(x, freqs_cis, wq, wk, wv, wo):
    full, _ = run(
        np.asarray(x, np.float32),
        np.asarray(freqs_cis, np.float32),
        np.asarray(wq, np.float32),
        np.asarray(wk, np.float32),
        np.asarray(wv, np.float32),
        np.asarray(wo, np.float32),
    )
    return full
